# revision 19
# baseline (speedup 1.0000x reference)
"""ECC eval-mode forward (nearest-prototype distance map) on 8 Trainium2 cores.

Reference:
    d2[n, m]  = ||x_n||^2 + ||p_m||^2 - 2 x_n . p_m      (n pixel, m of 48 protos)
    out[k, n] = max over the 8 prototypes of class k of sqrt(max(d2, 0))

Sharding: data-parallel, core i handles batch b = i//2, image rows
[128*(i%2), 128*(i%2)+128).  Per-core x slice is (C=512, 32768 px), streamed
as bf16 in 16 tiles of 2048 px.

Device pipeline per 1024-px PSUM batch:
  zp  (64,1024) = -2 x.p           M=64 matmuls; stationary column 32j+6p'+k
                                   holds proto p=4j+p' of class k
  zp += xsq[n]                     all-ones (128,64) stationary matmuls over
                                   x^2: reduces sum_c x^2 and broadcasts it
                                   into every psum row in one instruction
  zp += p_sq[m]                    free per-partition bias on the evictions
                                   (ScalarE activation Identity+bias);
                                   pad rows get psq = -1e30
  fold:    evict the two 32-row halves to two base-0 SBUF tiles and tt-max
           them (2-input DVE ops cannot read PSUM and need equal partition
           bases, and sub-32 partition moves are not expressible on this
           stack's engines) -> 24 rows: max over {p', p'+4} per (p', class)
  sqrt on ScalarE -> (24, 2048) fp32 -> contiguous DMA out

The remaining max over the four p' rows per class is an elementwise numpy
maximum folded into the host-side gather (sqrt is monotone, so max and sqrt
commute).
"""

import numpy as np
import ml_dtypes

import concourse.bacc as bacc
import concourse.tile as tile
from concourse import mybir
from concourse.bass_utils import run_bass_kernel_spmd

N_CORES = 8
C = 512
NCHUNK = 4
KCLS = 6
NPROTO = 8
NROW = 24                    # device output rows: 6p' + k
B, H, W = 4, 256, 256
NPIX = (H // 2) * W          # 32768 px per core
PTILE = 2048                 # pixels per DMA tile
NPT = NPIX // PTILE          # 16
BATCH = 1024                 # pixels per PSUM batch (2 banks of 512)
GS = 512                     # matmul free-dim group

F32 = mybir.dt.float32
BF16 = mybir.dt.bfloat16
AF = mybir.ActivationFunctionType
OP = mybir.AluOpType


def _build():
    nc = bacc.Bacc("TRN2", target_bir_lowering=False, debug=False,
                   num_devices=N_CORES)
    x_d = nc.dram_tensor("x", [C, NPIX], BF16, kind="ExternalInput")
    pT_d = nc.dram_tensor("protoT", [NCHUNK, 128, 64], BF16,
                          kind="ExternalInput")
    psq_d = nc.dram_tensor("psqbias", [2, 32], F32, kind="ExternalInput")
    out_d = nc.dram_tensor("out", [NROW, NPIX], F32, kind="ExternalOutput")

    with tile.TileContext(nc) as tc:
        with (
            tc.tile_pool(name="consts", bufs=1) as consts,
            tc.tile_pool(name="xin", bufs=4) as xin,
            tc.tile_pool(name="xsq", bufs=4) as xsqp,
            tc.tile_pool(name="lad", bufs=3) as lad,
            tc.tile_pool(name="ost", bufs=3) as ost,
            tc.tile_pool(name="zpp", bufs=4, space="PSUM") as zpp,
        ):
            pT = []
            for cc in range(NCHUNK):
                t = consts.tile([128, 64], BF16, tag=f"pT{cc}")
                nc.sync.dma_start(t[:], pT_d.ap()[cc])
                pT.append(t)
            psqA = consts.tile([32, 1], F32, tag="psqA")
            nc.sync.dma_start(psqA[:], psq_d.ap()[0].rearrange("(a b) -> a b", b=1))
            psqB = consts.tile([32, 1], F32, tag="psqB")
            nc.sync.dma_start(psqB[:], psq_d.ap()[1].rearrange("(a b) -> a b", b=1))
            allones = consts.tile([128, 64], BF16, tag="allones")
            nc.vector.memset(allones[:], 1.0)
            for pt in range(NPT):
                psl = slice(pt * PTILE, (pt + 1) * PTILE)
                xc = []
                for cc in range(NCHUNK):
                    t = xin.tile([128, PTILE], BF16, tag=f"x{cc}")
                    eng = nc.sync if cc % 2 == 0 else nc.scalar
                    eng.dma_start(
                        t[:], x_d.ap()[cc * 128:(cc + 1) * 128, psl])
                    xc.append(t)
                x2 = []
                for cc in range(NCHUNK):
                    t = xsqp.tile([128, PTILE], BF16, tag=f"x2{cc}")
                    if cc in (0, 1, 3):
                        nc.vector.tensor_mul(t[:], xc[cc][:], xc[cc][:])
                    else:
                        nc.gpsimd.tensor_mul(t[:], xc[cc][:], xc[cc][:])
                    x2.append(t)

                pre = ost.tile([32, PTILE], BF16, tag="pre")
                for hb in range(PTILE // BATCH):
                    bsl = slice(hb * BATCH, (hb + 1) * BATCH)
                    zp = zpp.tile([64, BATCH], F32, tag="zp")
                    for cc in range(NCHUNK):
                        for g in range(BATCH // GS):
                            goff = hb * BATCH + g * GS
                            gsl = slice(goff, goff + GS)     # in ptile tiles
                            qsl = slice(g * GS, (g + 1) * GS)  # in batch tiles
                            nc.tensor.matmul(
                                zp[:, qsl], pT[cc][:], xc[cc][:, gsl],
                                start=(cc == 0), stop=False)
                    for cc in range(NCHUNK):
                        for g in range(BATCH // GS):
                            goff = hb * BATCH + g * GS
                            gsl = slice(goff, goff + GS)
                            qsl = slice(g * GS, (g + 1) * GS)
                            nc.tensor.matmul(
                                zp[:, qsl], allones[:], x2[cc][:, gsl],
                                start=False, stop=(cc == NCHUNK - 1))

                    za = lad.tile([32, BATCH], BF16, tag="za")
                    nc.scalar.activation(za[:], zp[0:32, :], AF.Identity,
                                         bias=psqA[:])
                    zb = lad.tile([32, BATCH], BF16, tag="zb")
                    nc.scalar.activation(zb[:], zp[32:64, :], AF.Identity,
                                         bias=psqB[:])
                    nc.vector.tensor_tensor(pre[:, bsl], za[:], zb[:],
                                            op=OP.max)

                fin = ost.tile([NROW, PTILE], F32, tag="fin")
                nc.scalar.activation(fin[:], pre[0:NROW, :], AF.Sqrt)
                nc.scalar.dma_start(out_d.ap()[:, psl], fin[:])

    nc.compile()
    return nc


_cache = {}


def _get_nc():
    if "nc" not in _cache:
        _cache["nc"] = _build()
    return _cache["nc"]


def _prep_const_inputs(prototype):
    proto = np.asarray(prototype, dtype=np.float32)      # (6, 8, 512)
    protoT = np.zeros((NCHUNK, 128, 64), np.float32)
    psq64 = np.full(64, -1e30, np.float32)
    psq = np.sum(proto * proto, axis=2, dtype=np.float32)  # (6, 8)
    for j in range(2):
        for pp in range(4):
            p = 4 * j + pp
            for k in range(KCLS):
                col = 32 * j + 6 * pp + k
                protoT[:, :, col] = (-2.0 * proto[k, p]).reshape(NCHUNK, 128)
                psq64[col] = psq[k, p]
    return protoT.astype(ml_dtypes.bfloat16), psq64.reshape(2, 32).copy()


def kernel(x, gt, prototype):
    del gt  # unused by the reference computation
    nc = _get_nc()
    protoT, psqbias = _prep_const_inputs(prototype)
    x = np.asarray(x)
    in_maps = []
    for core in range(N_CORES):
        b, hh = divmod(core, 2)
        xs = np.ascontiguousarray(
            x[b, :, hh * 128:(hh + 1) * 128, :]).reshape(C, NPIX)
        in_maps.append({
            "x": xs.astype(ml_dtypes.bfloat16),
            "protoT": protoT, "psqbias": psqbias,
        })
    res = run_bass_kernel_spmd(nc, in_maps, core_ids=list(range(N_CORES)))
    out = np.empty((B, KCLS, H, W), np.float32)
    for core in range(N_CORES):
        b, hh = divmod(core, 2)
        r = res.results[core]["out"]           # (24, NPIX), rows 6p'+k
        r = r.reshape(4, KCLS, H // 2, W)      # [p', k, h, w]
        out[b, :, hh * 128:(hh + 1) * 128, :] = r.max(axis=0)
    return out


# revision 20
# speedup vs baseline: 1.1038x; 1.1038x over previous
"""ECC eval-mode forward (nearest-prototype distance map) on 8 Trainium2 cores.

Reference:
    d2[n, m]  = ||x_n||^2 + ||p_m||^2 - 2 x_n . p_m      (n pixel, m of 48 protos)
    out[k, n] = max over the 8 prototypes of class k of sqrt(max(d2, 0))

Sharding: data-parallel, core i handles batch b = i//2, image rows
[128*(i%2), 128*(i%2)+128).  Per-core x slice is (C=512, 32768 px), streamed
as bf16 in 16 tiles of 2048 px.

Device pipeline per 1024-px PSUM batch:
  zp  (64,1024) = -2 x.p           M=64 matmuls; stationary column 32j+6p'+k
                                   holds proto p=4j+p' of class k
  zp += xsq[n]                     all-ones (128,64) stationary matmuls over
                                   x^2: reduces sum_c x^2 and broadcasts it
                                   into every psum row in one instruction
  zp += p_sq[m]                    free per-partition bias on the evictions
                                   (ScalarE activation Identity+bias);
                                   pad rows get psq = -1e30
  fold:    evict the two 32-row halves to two base-0 SBUF tiles and tt-max
           them (2-input DVE ops cannot read PSUM and need equal partition
           bases, and sub-32 partition moves are not expressible on this
           stack's engines) -> 24 rows: max over {p', p'+4} per (p', class)
  sqrt on ScalarE -> (24, 2048) fp32 -> contiguous DMA out

The remaining max over the four p' rows per class is an elementwise numpy
maximum folded into the host-side gather (sqrt is monotone, so max and sqrt
commute).
"""

import numpy as np
import ml_dtypes

import concourse.bacc as bacc
import concourse.tile as tile
from concourse import mybir
from concourse.bass_utils import run_bass_kernel_spmd

N_CORES = 8
C = 512
NCHUNK = 4
KCLS = 6
NPROTO = 8
NROW = 24                    # device output rows: 6p' + k
B, H, W = 4, 256, 256
NPIX = (H // 2) * W          # 32768 px per core
PTILE = 2048                 # pixels per DMA tile
NPT = NPIX // PTILE          # 16
BATCH = 1024                 # pixels per PSUM batch (2 banks of 512)
GS = 512                     # matmul free-dim group

F32 = mybir.dt.float32
BF16 = mybir.dt.bfloat16
AF = mybir.ActivationFunctionType
OP = mybir.AluOpType


def _build():
    nc = bacc.Bacc("TRN2", target_bir_lowering=False, debug=False,
                   num_devices=N_CORES)
    x_d = nc.dram_tensor("x", [C, NPIX], BF16, kind="ExternalInput")
    pT_d = nc.dram_tensor("protoT", [NCHUNK, 128, 64], BF16,
                          kind="ExternalInput")
    psq_d = nc.dram_tensor("psqbias", [2, 32], F32, kind="ExternalInput")
    out_d = nc.dram_tensor("out", [NROW, NPIX], F32, kind="ExternalOutput")

    with tile.TileContext(nc) as tc:
        with (
            tc.tile_pool(name="consts", bufs=1) as consts,
            tc.tile_pool(name="xin", bufs=4) as xin,
            tc.tile_pool(name="xsq", bufs=4) as xsqp,
            tc.tile_pool(name="lad", bufs=3) as lad,
            tc.tile_pool(name="ost", bufs=3) as ost,
            tc.tile_pool(name="zpp", bufs=4, space="PSUM") as zpp,
        ):
            pT = []
            for cc in range(NCHUNK):
                t = consts.tile([128, 64], BF16, tag=f"pT{cc}")
                nc.sync.dma_start(t[:], pT_d.ap()[cc])
                pT.append(t)
            psqA = consts.tile([32, 1], F32, tag="psqA")
            nc.sync.dma_start(psqA[:], psq_d.ap()[0].rearrange("(a b) -> a b", b=1))
            psqB = consts.tile([32, 1], F32, tag="psqB")
            nc.sync.dma_start(psqB[:], psq_d.ap()[1].rearrange("(a b) -> a b", b=1))
            allones = consts.tile([128, 64], BF16, tag="allones")
            nc.vector.memset(allones[:], 1.0)
            for pt in range(NPT):
                psl = slice(pt * PTILE, (pt + 1) * PTILE)
                xc = []
                for cc in range(NCHUNK):
                    t = xin.tile([128, PTILE], BF16, tag=f"x{cc}")
                    nc.sync.dma_start(
                        t[:], x_d.ap()[cc * 128:(cc + 1) * 128, psl])
                    xc.append(t)
                x2 = []
                for cc in range(NCHUNK):
                    t = xsqp.tile([128, PTILE], BF16, tag=f"x2{cc}")
                    if cc in (0, 1, 3):
                        nc.vector.tensor_mul(t[:], xc[cc][:], xc[cc][:])
                    else:
                        nc.gpsimd.tensor_mul(t[:], xc[cc][:], xc[cc][:])
                    x2.append(t)

                pre = ost.tile([32, PTILE], BF16, tag="pre")
                for hb in range(PTILE // BATCH):
                    bsl = slice(hb * BATCH, (hb + 1) * BATCH)
                    zp = zpp.tile([64, BATCH], F32, tag="zp")
                    for cc in range(NCHUNK):
                        for g in range(BATCH // GS):
                            goff = hb * BATCH + g * GS
                            gsl = slice(goff, goff + GS)     # in ptile tiles
                            qsl = slice(g * GS, (g + 1) * GS)  # in batch tiles
                            nc.tensor.matmul(
                                zp[:, qsl], pT[cc][:], xc[cc][:, gsl],
                                start=(cc == 0), stop=False)
                    for cc in range(NCHUNK):
                        for g in range(BATCH // GS):
                            goff = hb * BATCH + g * GS
                            gsl = slice(goff, goff + GS)
                            qsl = slice(g * GS, (g + 1) * GS)
                            nc.tensor.matmul(
                                zp[:, qsl], allones[:], x2[cc][:, gsl],
                                start=False, stop=(cc == NCHUNK - 1))

                    za = lad.tile([32, BATCH], BF16, tag="za")
                    nc.scalar.activation(za[:], zp[0:32, :], AF.Identity,
                                         bias=psqA[:])
                    zb = lad.tile([32, BATCH], BF16, tag="zb")
                    nc.scalar.activation(zb[:], zp[32:64, :], AF.Identity,
                                         bias=psqB[:])
                    nc.vector.tensor_tensor(pre[:, bsl], za[:], zb[:],
                                            op=OP.max)

                fin = ost.tile([NROW, PTILE], F32, tag="fin")
                nc.scalar.activation(fin[:], pre[0:NROW, :], AF.Sqrt)
                nc.sync.dma_start(out_d.ap()[:, psl], fin[:])

    nc.compile()
    return nc


_cache = {}


def _get_nc():
    if "nc" not in _cache:
        _cache["nc"] = _build()
    return _cache["nc"]


def _prep_const_inputs(prototype):
    proto = np.asarray(prototype, dtype=np.float32)      # (6, 8, 512)
    protoT = np.zeros((NCHUNK, 128, 64), np.float32)
    psq64 = np.full(64, -1e30, np.float32)
    psq = np.sum(proto * proto, axis=2, dtype=np.float32)  # (6, 8)
    for j in range(2):
        for pp in range(4):
            p = 4 * j + pp
            for k in range(KCLS):
                col = 32 * j + 6 * pp + k
                protoT[:, :, col] = (-2.0 * proto[k, p]).reshape(NCHUNK, 128)
                psq64[col] = psq[k, p]
    return protoT.astype(ml_dtypes.bfloat16), psq64.reshape(2, 32).copy()


def kernel(x, gt, prototype):
    del gt  # unused by the reference computation
    nc = _get_nc()
    protoT, psqbias = _prep_const_inputs(prototype)
    x = np.asarray(x)
    in_maps = []
    for core in range(N_CORES):
        b, hh = divmod(core, 2)
        xs = np.ascontiguousarray(
            x[b, :, hh * 128:(hh + 1) * 128, :]).reshape(C, NPIX)
        in_maps.append({
            "x": xs.astype(ml_dtypes.bfloat16),
            "protoT": protoT, "psqbias": psqbias,
        })
    res = run_bass_kernel_spmd(nc, in_maps, core_ids=list(range(N_CORES)))
    out = np.empty((B, KCLS, H, W), np.float32)
    for core in range(N_CORES):
        b, hh = divmod(core, 2)
        r = res.results[core]["out"]           # (24, NPIX), rows 6p'+k
        r = r.reshape(4, KCLS, H // 2, W)      # [p', k, h, w]
        out[b, :, hh * 128:(hh + 1) * 128, :] = r.max(axis=0)
    return out


# revision 21
# speedup vs baseline: 1.1066x; 1.0025x over previous
"""ECC eval-mode forward (nearest-prototype distance map) on 8 Trainium2 cores.

Reference:
    d2[n, m]  = ||x_n||^2 + ||p_m||^2 - 2 x_n . p_m      (n pixel, m of 48 protos)
    out[k, n] = max over the 8 prototypes of class k of sqrt(max(d2, 0))

Sharding: data-parallel, core i handles batch b = i//2, image rows
[128*(i%2), 128*(i%2)+128).  Per-core x slice is (C=512, 32768 px), streamed
as bf16 in 16 tiles of 2048 px.

Device pipeline per 1024-px PSUM batch:
  zp  (64,1024) = -2 x.p           M=64 matmuls; stationary column 32j+6p'+k
                                   holds proto p=4j+p' of class k
  zp += xsq[n]                     all-ones (128,64) stationary matmuls over
                                   x^2: reduces sum_c x^2 and broadcasts it
                                   into every psum row in one instruction
  zp += p_sq[m]                    free per-partition bias on the evictions
                                   (ScalarE activation Identity+bias);
                                   pad rows get psq = -1e30
  fold:    evict the two 32-row halves to two base-0 SBUF tiles and tt-max
           them (2-input DVE ops cannot read PSUM and need equal partition
           bases, and sub-32 partition moves are not expressible on this
           stack's engines) -> 24 rows: max over {p', p'+4} per (p', class)
  sqrt on ScalarE -> (24, 2048) fp32 -> contiguous DMA out

The remaining max over the four p' rows per class is an elementwise numpy
maximum folded into the host-side gather (sqrt is monotone, so max and sqrt
commute).
"""

import numpy as np
import ml_dtypes

import concourse.bacc as bacc
import concourse.tile as tile
from concourse import mybir
from concourse.bass_utils import run_bass_kernel_spmd

N_CORES = 8
C = 512
NCHUNK = 4
KCLS = 6
NPROTO = 8
NROW = 24                    # device output rows: 6p' + k
B, H, W = 4, 256, 256
NPIX = (H // 2) * W          # 32768 px per core
PTILE = 2048                 # pixels per DMA tile
NPT = NPIX // PTILE          # 16
BATCH = 1024                 # pixels per PSUM batch (2 banks of 512)
GS = 512                     # matmul free-dim group

F32 = mybir.dt.float32
BF16 = mybir.dt.bfloat16
AF = mybir.ActivationFunctionType
OP = mybir.AluOpType


def _build():
    nc = bacc.Bacc("TRN2", target_bir_lowering=False, debug=False,
                   num_devices=N_CORES)
    x_d = nc.dram_tensor("x", [C, NPIX], BF16, kind="ExternalInput")
    pT_d = nc.dram_tensor("protoT", [NCHUNK, 128, 64], BF16,
                          kind="ExternalInput")
    psq_d = nc.dram_tensor("psqbias", [2, 32], F32, kind="ExternalInput")
    out_d = nc.dram_tensor("out", [NROW, NPIX], F32, kind="ExternalOutput")

    with tile.TileContext(nc) as tc:
        with (
            tc.tile_pool(name="consts", bufs=1) as consts,
            tc.tile_pool(name="xin", bufs=5) as xin,
            tc.tile_pool(name="xsq", bufs=3) as xsqp,
            tc.tile_pool(name="lad", bufs=3) as lad,
            tc.tile_pool(name="ost", bufs=3) as ost,
            tc.tile_pool(name="zpp", bufs=4, space="PSUM") as zpp,
        ):
            pT = []
            for cc in range(NCHUNK):
                t = consts.tile([128, 64], BF16, tag=f"pT{cc}")
                nc.sync.dma_start(t[:], pT_d.ap()[cc])
                pT.append(t)
            psqA = consts.tile([32, 1], F32, tag="psqA")
            nc.sync.dma_start(psqA[:], psq_d.ap()[0].rearrange("(a b) -> a b", b=1))
            psqB = consts.tile([32, 1], F32, tag="psqB")
            nc.sync.dma_start(psqB[:], psq_d.ap()[1].rearrange("(a b) -> a b", b=1))
            allones = consts.tile([128, 64], BF16, tag="allones")
            nc.vector.memset(allones[:], 1.0)
            for pt in range(NPT):
                psl = slice(pt * PTILE, (pt + 1) * PTILE)
                xc = []
                for cc in range(NCHUNK):
                    t = xin.tile([128, PTILE], BF16, tag=f"x{cc}")
                    nc.sync.dma_start(
                        t[:], x_d.ap()[cc * 128:(cc + 1) * 128, psl])
                    xc.append(t)
                x2 = []
                for cc in range(NCHUNK):
                    t = xsqp.tile([128, PTILE], BF16, tag=f"x2{cc}")
                    if cc in (0, 1, 3):
                        nc.vector.tensor_mul(t[:], xc[cc][:], xc[cc][:])
                    else:
                        nc.gpsimd.tensor_mul(t[:], xc[cc][:], xc[cc][:])
                    x2.append(t)

                pre = ost.tile([32, PTILE], BF16, tag="pre")
                for hb in range(PTILE // BATCH):
                    bsl = slice(hb * BATCH, (hb + 1) * BATCH)
                    zp = zpp.tile([64, BATCH], F32, tag="zp")
                    for cc in range(NCHUNK):
                        for g in range(BATCH // GS):
                            goff = hb * BATCH + g * GS
                            gsl = slice(goff, goff + GS)     # in ptile tiles
                            qsl = slice(g * GS, (g + 1) * GS)  # in batch tiles
                            nc.tensor.matmul(
                                zp[:, qsl], pT[cc][:], xc[cc][:, gsl],
                                start=(cc == 0), stop=False)
                    for cc in range(NCHUNK):
                        for g in range(BATCH // GS):
                            goff = hb * BATCH + g * GS
                            gsl = slice(goff, goff + GS)
                            qsl = slice(g * GS, (g + 1) * GS)
                            nc.tensor.matmul(
                                zp[:, qsl], allones[:], x2[cc][:, gsl],
                                start=False, stop=(cc == NCHUNK - 1))

                    za = lad.tile([32, BATCH], BF16, tag="za")
                    nc.scalar.activation(za[:], zp[0:32, :], AF.Identity,
                                         bias=psqA[:])
                    zb = lad.tile([32, BATCH], BF16, tag="zb")
                    nc.scalar.activation(zb[:], zp[32:64, :], AF.Identity,
                                         bias=psqB[:])
                    nc.vector.tensor_tensor(pre[:, bsl], za[:], zb[:],
                                            op=OP.max)

                fin = ost.tile([NROW, PTILE], F32, tag="fin")
                nc.scalar.activation(fin[:], pre[0:NROW, :], AF.Sqrt)
                nc.sync.dma_start(out_d.ap()[:, psl], fin[:])

    nc.compile()
    return nc


_cache = {}


def _get_nc():
    if "nc" not in _cache:
        _cache["nc"] = _build()
    return _cache["nc"]


def _prep_const_inputs(prototype):
    proto = np.asarray(prototype, dtype=np.float32)      # (6, 8, 512)
    protoT = np.zeros((NCHUNK, 128, 64), np.float32)
    psq64 = np.full(64, -1e30, np.float32)
    psq = np.sum(proto * proto, axis=2, dtype=np.float32)  # (6, 8)
    for j in range(2):
        for pp in range(4):
            p = 4 * j + pp
            for k in range(KCLS):
                col = 32 * j + 6 * pp + k
                protoT[:, :, col] = (-2.0 * proto[k, p]).reshape(NCHUNK, 128)
                psq64[col] = psq[k, p]
    return protoT.astype(ml_dtypes.bfloat16), psq64.reshape(2, 32).copy()


def kernel(x, gt, prototype):
    del gt  # unused by the reference computation
    nc = _get_nc()
    protoT, psqbias = _prep_const_inputs(prototype)
    x = np.asarray(x)
    in_maps = []
    for core in range(N_CORES):
        b, hh = divmod(core, 2)
        xs = np.ascontiguousarray(
            x[b, :, hh * 128:(hh + 1) * 128, :]).reshape(C, NPIX)
        in_maps.append({
            "x": xs.astype(ml_dtypes.bfloat16),
            "protoT": protoT, "psqbias": psqbias,
        })
    res = run_bass_kernel_spmd(nc, in_maps, core_ids=list(range(N_CORES)))
    out = np.empty((B, KCLS, H, W), np.float32)
    for core in range(N_CORES):
        b, hh = divmod(core, 2)
        r = res.results[core]["out"]           # (24, NPIX), rows 6p'+k
        r = r.reshape(4, KCLS, H // 2, W)      # [p', k, h, w]
        out[b, :, hh * 128:(hh + 1) * 128, :] = r.max(axis=0)
    return out


# revision 22
# speedup vs baseline: 1.2054x; 1.0893x over previous
"""ECC eval-mode forward (nearest-prototype distance map) on 8 Trainium2 cores.

Reference:
    d2[n, m]  = ||x_n||^2 + ||p_m||^2 - 2 x_n . p_m      (n pixel, m of 48 protos)
    out[k, n] = max over the 8 prototypes of class k of sqrt(max(d2, 0))

Sharding: data-parallel, core i handles batch b = i//2, image rows
[128*(i%2), 128*(i%2)+128).  Per-core x slice is (C=512, 32768 px), streamed
as bf16 in 16 tiles of 2048 px.

Device pipeline per 1024-px PSUM batch:
  zp  (64,1024) = -2 x.p           M=64 matmuls; stationary column 32j+6p'+k
                                   holds proto p=4j+p' of class k
  zp += xsq[n]                     all-ones (128,64) stationary matmuls over
                                   x^2: reduces sum_c x^2 and broadcasts it
                                   into every psum row in one instruction
  zp += p_sq[m]                    free per-partition bias on the evictions
                                   (ScalarE activation Identity+bias);
                                   pad rows get psq = -1e30
  fold:    evict the two 32-row halves to two base-0 SBUF tiles and tt-max
           them (2-input DVE ops cannot read PSUM and need equal partition
           bases, and sub-32 partition moves are not expressible on this
           stack's engines) -> 24 rows: max over {p', p'+4} per (p', class)
  sqrt on ScalarE -> (24, 2048) fp32 -> contiguous DMA out

The remaining max over the four p' rows per class is an elementwise numpy
maximum folded into the host-side gather (sqrt is monotone, so max and sqrt
commute).
"""

import numpy as np
import ml_dtypes

import concourse.bacc as bacc
import concourse.tile as tile
from concourse import mybir
from concourse.bass_utils import run_bass_kernel_spmd

N_CORES = 8
C = 512
NCHUNK = 4
KCLS = 6
NPROTO = 8
NROW = 24                    # device output rows: 6p' + k
B, H, W = 4, 256, 256
NPIX = (H // 2) * W          # 32768 px per core
PTILE = 2048                 # pixels per DMA tile
NPT = NPIX // PTILE          # 16
BATCH = 1024                 # pixels per PSUM batch (2 banks of 512)
GS = 512                     # matmul free-dim group

F32 = mybir.dt.float32
BF16 = mybir.dt.bfloat16
AF = mybir.ActivationFunctionType
OP = mybir.AluOpType


def _build():
    nc = bacc.Bacc("TRN2", target_bir_lowering=False, debug=False,
                   num_devices=N_CORES)
    x_d = nc.dram_tensor("x", [C, NPIX], BF16, kind="ExternalInput")
    pT_d = nc.dram_tensor("protoT", [NCHUNK, 128, 64], BF16,
                          kind="ExternalInput")
    psq_d = nc.dram_tensor("psqbias", [2, 32], F32, kind="ExternalInput")
    out_d = nc.dram_tensor("out", [NROW, NPIX], F32, kind="ExternalOutput")

    with tile.TileContext(nc) as tc:
        with (
            tc.tile_pool(name="consts", bufs=1) as consts,
            tc.tile_pool(name="xin", bufs=5) as xin,
            tc.tile_pool(name="xsq", bufs=3) as xsqp,
            tc.tile_pool(name="lad", bufs=3) as lad,
            tc.tile_pool(name="ost", bufs=3) as ost,
            tc.tile_pool(name="zpp", bufs=4, space="PSUM") as zpp,
        ):
            pT = []
            for cc in range(NCHUNK):
                t = consts.tile([128, 64], BF16, tag=f"pT{cc}")
                nc.sync.dma_start(t[:], pT_d.ap()[cc])
                pT.append(t)
            psqA = consts.tile([32, 1], F32, tag="psqA")
            nc.sync.dma_start(psqA[:], psq_d.ap()[0].rearrange("(a b) -> a b", b=1))
            psqB = consts.tile([32, 1], F32, tag="psqB")
            nc.sync.dma_start(psqB[:], psq_d.ap()[1].rearrange("(a b) -> a b", b=1))
            allones = consts.tile([128, 64], BF16, tag="allones")
            nc.vector.memset(allones[:], 1.0)
            for pt in range(NPT):
                psl = slice(pt * PTILE, (pt + 1) * PTILE)
                xc = []
                for cc in range(NCHUNK):
                    t = xin.tile([128, PTILE], BF16, tag=f"x{cc}")
                    nc.sync.dma_start(
                        t[:], x_d.ap()[cc * 128:(cc + 1) * 128, psl])
                    xc.append(t)
                x2 = []
                for cc in range(NCHUNK):
                    t = xsqp.tile([128, PTILE], BF16, tag=f"x2{cc}")
                    if cc in (0, 1, 3):
                        nc.vector.tensor_mul(t[:], xc[cc][:], xc[cc][:])
                    else:
                        nc.gpsimd.tensor_mul(t[:], xc[cc][:], xc[cc][:])
                    x2.append(t)

                pre = ost.tile([32, PTILE], BF16, tag="pre")
                for hb in range(PTILE // BATCH):
                    bsl = slice(hb * BATCH, (hb + 1) * BATCH)
                    zp = zpp.tile([64, BATCH], F32, tag="zp")
                    for cc in range(NCHUNK):
                        for g in range(BATCH // GS):
                            goff = hb * BATCH + g * GS
                            gsl = slice(goff, goff + GS)     # in ptile tiles
                            qsl = slice(g * GS, (g + 1) * GS)  # in batch tiles
                            nc.tensor.matmul(
                                zp[:, qsl], pT[cc][:], xc[cc][:, gsl],
                                start=(cc == 0), stop=False)
                    for cc in range(NCHUNK):
                        for g in range(BATCH // GS):
                            goff = hb * BATCH + g * GS
                            gsl = slice(goff, goff + GS)
                            qsl = slice(g * GS, (g + 1) * GS)
                            nc.tensor.matmul(
                                zp[:, qsl], allones[:], x2[cc][:, gsl],
                                start=False, stop=(cc == NCHUNK - 1))

                    za = lad.tile([32, BATCH], BF16, tag="za")
                    nc.scalar.activation(za[:], zp[0:32, :], AF.Identity,
                                         bias=psqA[:])
                    zb = lad.tile([32, BATCH], BF16, tag="zb")
                    nc.scalar.activation(zb[:], zp[32:64, :], AF.Identity,
                                         bias=psqB[:])
                    nc.vector.tensor_tensor(pre[:, bsl], za[:], zb[:],
                                            op=OP.max)

                fin = ost.tile([NROW, PTILE], F32, tag="fin")
                nc.scalar.activation(fin[:], pre[0:NROW, :], AF.Sqrt)
                nc.gpsimd.dma_start(out_d.ap()[:, psl], fin[:])

    nc.compile()
    return nc


_cache = {}


def _get_nc():
    if "nc" not in _cache:
        _cache["nc"] = _build()
    return _cache["nc"]


def _prep_const_inputs(prototype):
    proto = np.asarray(prototype, dtype=np.float32)      # (6, 8, 512)
    protoT = np.zeros((NCHUNK, 128, 64), np.float32)
    psq64 = np.full(64, -1e30, np.float32)
    psq = np.sum(proto * proto, axis=2, dtype=np.float32)  # (6, 8)
    for j in range(2):
        for pp in range(4):
            p = 4 * j + pp
            for k in range(KCLS):
                col = 32 * j + 6 * pp + k
                protoT[:, :, col] = (-2.0 * proto[k, p]).reshape(NCHUNK, 128)
                psq64[col] = psq[k, p]
    return protoT.astype(ml_dtypes.bfloat16), psq64.reshape(2, 32).copy()


def kernel(x, gt, prototype):
    del gt  # unused by the reference computation
    nc = _get_nc()
    protoT, psqbias = _prep_const_inputs(prototype)
    x = np.asarray(x)
    in_maps = []
    for core in range(N_CORES):
        b, hh = divmod(core, 2)
        xs = np.ascontiguousarray(
            x[b, :, hh * 128:(hh + 1) * 128, :]).reshape(C, NPIX)
        in_maps.append({
            "x": xs.astype(ml_dtypes.bfloat16),
            "protoT": protoT, "psqbias": psqbias,
        })
    res = run_bass_kernel_spmd(nc, in_maps, core_ids=list(range(N_CORES)))
    out = np.empty((B, KCLS, H, W), np.float32)
    for core in range(N_CORES):
        b, hh = divmod(core, 2)
        r = res.results[core]["out"]           # (24, NPIX), rows 6p'+k
        r = r.reshape(4, KCLS, H // 2, W)      # [p', k, h, w]
        out[b, :, hh * 128:(hh + 1) * 128, :] = r.max(axis=0)
    return out
